# revision 1
# baseline (speedup 1.0000x reference)
"""MLA-style causal self-attention on 8 Trainium2 NeuronCores.

Sharding: tensor-parallel over heads (2 heads/core). W_qdec column-split,
W_out row-split by head; x / W_qkv / rope tables replicated. Each core
returns a partial out^T [E, T]; the host sums the 8 partials (the standard
gather for row-split tensor parallelism) and transposes.

Device dataflow (everything kept "transposed" so matmuls contract over the
partition dim with no activation transposes except c_kv -> v):
  A:    latents^T = W_qkv^T @ x^T, 13 output-column groups per 256-wide
        T chunk (kv+rope -> SBUF residents, c_q -> DRAM scratch).
  preC: rope k_r^T (rotate-half via a 64x64 permutation matmul, so no
        DMA sits on the critical path); v = (c_kv^T)^T via PE transposes.
  B/C fused per head h and 512-query chunk i4:
        q^T chunk = W_qdec_h^T @ c_q^T chunk (scale folded in), rope
        group first so the rope chain overlaps the nope groups, then
        flash-style causal attention: st[k,q] accumulated over 5 key
        matmuls, exp on ScalarE, edge-masked, denominator via
        ones-matmul over a DVE-accumulated sum, y^T[d,q] accumulated in
        4 PSUM banks, normalized, staged to per-chunk DRAM tiles.
  D:    out^T = W_out_c^T @ y^T -> HBM (kc-major so the stationary
        operand is reused across 4 consecutive matmuls).
All big matmuls run in float32r (full PE rate, fp32 PSUM accumulate).
"""

import math
from contextlib import ExitStack

import numpy as np

import concourse.bass as bass
import concourse.tile as tile
from concourse import bacc, mybir
from concourse.bass_utils import run_bass_kernel_spmd
from concourse.masks import make_identity

F32 = mybir.dt.float32
F32R = mybir.dt.float32r
AF = mybir.ActivationFunctionType

# Problem constants (hardcoded per harness contract)
T_FULL = 2048
E = 2048          # n_embd
KV = 512          # kv low rank == head size
QL = 1024         # q low rank
RH = 64           # rope head size
QKH = KV + RH     # 576
NH = 16
NCORES = 8
HPC = NH // NCORES  # heads per core
SCALE = 1.0 / math.sqrt(float(KV))

P = 128


def _make_rot64(nc, pool):
    """RT [64, 64] permutation with RT[x, y] = 1 iff x == (y+32) % 64, so
    matmul(out, lhsT=RT, rhs=src) gives out[d] = src[(d+32) % 64]."""
    rt0 = pool.tile([RH, RH], F32, tag="rt0")
    nc.gpsimd.memset(rt0[:], 0.0)
    # fill 1 where x - y - 32 == 0
    nc.gpsimd.affine_select(
        out=rt0[:], in_=rt0[:], compare_op=mybir.AluOpType.not_equal,
        fill=1.0, base=-32, channel_multiplier=1, pattern=[[-1, RH]],
    )
    # fill 1 where x - y + 32 == 0
    nc.gpsimd.affine_select(
        out=rt0[:], in_=rt0[:], compare_op=mybir.AluOpType.not_equal,
        fill=1.0, base=32, channel_multiplier=1, pattern=[[-1, RH]],
    )
    rt = pool.tile([RH, RH], F32R, tag="rt")
    nc.vector.tensor_copy(rt[:], rt0[:])
    return rt


def build_kernel(T=T_FULL):
    """Build the single-core program (SPMD across 8 cores via per-core data)."""
    assert T % 512 == 0
    NT512 = T // 512
    NT256 = T // 256
    NKT = T // P          # key tiles of 128
    EK = E // P           # 16 contraction chunks for phase A

    nc = bacc.Bacc("TRN2", target_bir_lowering=False, debug=False,
                   num_devices=NCORES)

    xT = nc.dram_tensor("xT", [E, T], F32R, kind="ExternalInput").ap()
    wqkv = nc.dram_tensor("wqkv", [E, QKH + QL], F32R, kind="ExternalInput").ap()
    wqdec = nc.dram_tensor("wqdec", [QL, HPC * QKH], F32R, kind="ExternalInput").ap()
    wout = nc.dram_tensor("wout", [HPC * KV, E], F32R, kind="ExternalInput").ap()
    cosd = nc.dram_tensor("cosT", [RH, T], F32, kind="ExternalInput").ap()
    sind = nc.dram_tensor("sinT", [RH, T], F32, kind="ExternalInput").ap()
    outT = nc.dram_tensor("outT", [E, T], F32, kind="ExternalOutput").ap()

    xT_r = xT.rearrange("(ko p) t -> p ko t", p=P)
    wq_r = wqkv.rearrange("(ko p) m -> p ko m", p=P)

    # latent column groups: 4x128 c_kv, 1x64 k_r, 8x128 c_q
    groups = [(i * P, P) for i in range(KV // P)] + [(KV, RH)] + [
        (QKH + i * P, P) for i in range(QL // P)
    ]

    with tile.TileContext(nc) as tc, ExitStack() as ctx:
        dram = ctx.enter_context(tc.tile_pool(name="dram", bufs=1, space="DRAM"))
        cst = ctx.enter_context(tc.tile_pool(name="cst", bufs=1))
        kvp = ctx.enter_context(tc.tile_pool(name="kvp", bufs=1))
        pp = ctx.enter_context(tc.tile_pool(name="pp", bufs=2, space="PSUM"))
        ppy = ctx.enter_context(tc.tile_pool(name="ppy", bufs=1, space="PSUM"))

        cqd = dram.tile([QL, T], F32R, tag="cqd")        # c_q^T scratch
        # y^T scratch, one DRAM tile per (head, d-chunk) so phase D can
        # stream each back as soon as that head's chunk is complete
        yTd = [dram.tile([P, T], F32R, tag=f"yTd{k}", name=f"yTd{k}")
               for k in range(HPC * KV // P)]
        cq_r = cqd[:].rearrange("(ko p) t -> p ko t", p=P)

        # ---- global constants / kv residents ----
        ident0 = cst.tile([P, P], F32, tag="ident0")
        make_identity(nc, ident0[:])
        ident = cst.tile([P, P], F32R, tag="ident")
        nc.vector.tensor_copy(ident[:], ident0[:])
        ones0 = cst.tile([P, 1], F32, tag="ones0")
        nc.gpsimd.memset(ones0[:], 1.0)
        ones_col = cst.tile([P, 1], F32R, tag="ones")
        nc.vector.tensor_copy(ones_col[:], ones0[:])

        ckvT = kvp.tile([P, KV // P, T], F32R, tag="ckvT")  # c_kv^T
        krT = kvp.tile([RH, T], F32R, tag="krT")            # k_r^T (pre-rope)
        krT2 = kvp.tile([RH, T], F32R, tag="krT2")          # k_r^T (roped)

        # ================= Phase A: latents^T = W_qkv^T @ x^T ==============
        with ExitStack() as actx:
            aw = actx.enter_context(tc.tile_pool(name="aw", bufs=1))
            asr = actx.enter_context(tc.tile_pool(name="asr", bufs=2))
            astp = actx.enter_context(tc.tile_pool(name="astp", bufs=3))

            EH = EK // 2
            wqt = []
            for gi, (c0, M) in enumerate(groups):
                wa = aw.tile([P, EH, M], F32R, tag=f"wqa{gi}", name=f"wqa{gi}")
                wb = aw.tile([P, EH, M], F32R, tag=f"wqb{gi}", name=f"wqb{gi}")
                e0, e1 = ((nc.gpsimd, nc.scalar) if gi % 2 == 0
                          else (nc.scalar, nc.gpsimd))
                if gi % 3 == 1:
                    # borrow the lightly-loaded sync queue for a third of
                    # the second halves: the first-chunk group consumption
                    # outruns two queues during the cold ramp
                    e1 = nc.sync
                e0.dma_start(wa[:], wq_r[:, 0:EH, c0 : c0 + M])
                e1.dma_start(wb[:], wq_r[:, EH:EK, c0 : c0 + M])
                wqt.append((wa, wb))

            for tcc in range(NT256):
                tsl = slice(tcc * 256, (tcc + 1) * 256)
                xta = asr.tile([P, EH, 256], F32R, tag="xta", name="xta")
                nc.sync.dma_start(xta[:], xT_r[:, 0:EH, tsl])
                xtb = asr.tile([P, EH, 256], F32R, tag="xtb", name="xtb")
                nc.sync.dma_start(xtb[:], xT_r[:, EH:EK, tsl])
                for gi, (c0, M) in enumerate(groups):
                    ps = pp.tile([P, 256], F32, tag="mm", name="psA")
                    for kc in range(EK):
                        xs = xta[:, kc, :] if kc < EH else xtb[:, kc - EH, :]
                        ws = (wqt[gi][0][:, kc, :] if kc < EH
                              else wqt[gi][1][:, kc - EH, :])
                        nc.tensor.matmul(
                            ps[:M], ws, xs,
                            start=(kc == 0), stop=(kc == EK - 1),
                        )
                    if c0 < KV:
                        nc.vector.tensor_copy(ckvT[:, c0 // P, tsl], ps[:])
                    elif c0 == KV:
                        nc.vector.tensor_copy(krT[:, tsl], ps[:RH])
                    else:
                        st = astp.tile([P, 256], F32R, tag="ast", name="ast")
                        nc.vector.tensor_copy(st[:], ps[:])
                        nc.scalar.dma_start(
                            cqd[c0 - QKH : c0 - QKH + M, tsl], st[:]
                        )

        # ============ Phases preC + fused B/C =============================
        with ExitStack() as bctx:
            bcp = bctx.enter_context(tc.tile_pool(name="bcp", bufs=1))
            bcs = bctx.enter_context(tc.tile_pool(name="bcs", bufs=2))

            # BC-scope constants: rope tables, rotation matrix, edge masks
            rt = _make_rot64(nc, bcp)
            masks = []
            for r in range(4):
                m0 = bcp.tile([P, 512], F32, tag=f"mask{r}", name=f"mask{r}")
                nc.gpsimd.memset(m0[:], 1.0)
                nc.gpsimd.affine_select(
                    out=m0[:], in_=m0[:], compare_op=mybir.AluOpType.is_ge,
                    fill=0.0, base=-P * r, channel_multiplier=-1,
                    pattern=[[1, 512]],
                )
                masks.append(m0)
            cosT = bcp.tile([RH, T], F32, tag="cosT")
            nc.sync.dma_start(cosT[:], cosd[:])
            ssinT = bcp.tile([RH, T], F32, tag="ssinT")
            nc.sync.dma_start(ssinT[:], sind[:])
            nc.vector.tensor_scalar_mul(
                ssinT[0 : RH // 2, :], ssinT[0 : RH // 2, :], -1.0
            )

            # rope k_r via permutation matmul, chunked
            for tcc in range(NT512):
                tsl = slice(tcc * 512, (tcc + 1) * 512)
                pr = pp.tile([RH, 512], F32, tag="mm", name="prk")
                nc.tensor.matmul(pr[:], rt[:], krT[:, tsl],
                                 start=True, stop=True)
                nc.vector.tensor_mul(krT2[:, tsl], krT[:, tsl], cosT[:, tsl])
                rot = bcs.tile([RH, 512], F32, tag="rot", name="rotk")
                nc.vector.tensor_mul(rot[:], pr[:], ssinT[:, tsl])
                nc.vector.tensor_add(krT2[:, tsl], krT2[:, tsl], rot[:])

            # v[t, d] via PE transposes of c_kv^T
            v = bcp.tile([P, NKT, KV], F32R, tag="v")
            with tc.tile_pool(name="ptr", bufs=2, space="PSUM") as ptr:
                for dc in range(KV // P):
                    for tt in range(NKT):
                        pt = ptr.tile([P, P], F32R, tag="tr", name="pt")
                        nc.tensor.transpose(
                            pt[:], ckvT[:, dc, tt * P : (tt + 1) * P], ident[:]
                        )
                        nc.vector.tensor_copy(
                            v[:, tt, dc * P : (dc + 1) * P], pt[:]
                        )
            pden = bctx.enter_context(
                tc.tile_pool(name="pden", bufs=2, space="PSUM")
            )

            # rope group first so its chain overlaps the nope matmul groups
            qgroups = [(KV, RH)] + [(i * P, P) for i in range(KV // P)]
            for h in range(HPC):
                wqd = bcp.tile([P, QL // P, QKH], F32R, tag="wqd", name="wqd")
                nc.gpsimd.dma_start(
                    wqd[:],
                    wqdec.rearrange("(ko p) m -> p ko m", p=P)[
                        :, :, h * QKH : (h + 1) * QKH
                    ],
                )
                for i4 in range(NT512):
                    qsl = slice(i4 * 512, (i4 + 1) * 512)
                    # ---- B chunk: q^T for queries i4 (SCALE folded in) ----
                    cq = bcs.tile([P, QL // P, 512], F32R, tag="cq", bufs=1,
                                  name="cq")
                    nc.sync.dma_start(cq[:], cq_r[:, :, qsl])
                    qTc = [bcs.tile([P, 512], F32R, tag=f"qTc{i}",
                                    name=f"qTc{i}") for i in range(KV // P)]
                    qrRaw = bcs.tile([RH, 512], F32R, tag="qrRaw", name="qrRaw")
                    qrT = bcs.tile([RH, 512], F32R, tag="qrT", name="qrT")
                    for (m0, M) in qgroups:
                        ps = pp.tile([P, 512], F32, tag="mm", name="psB")
                        for kc in range(QL // P):
                            nc.tensor.matmul(
                                ps[:M], wqd[:, kc, m0 : m0 + M], cq[:, kc, :],
                                start=(kc == 0), stop=(kc == QL // P - 1),
                            )
                        if m0 < KV:
                            nc.vector.tensor_scalar_mul(
                                qTc[m0 // P][:], ps[:], SCALE
                            )
                        else:
                            # rope group runs first; do its whole chain now so
                            # the DVE work hides under the four d-groups
                            nc.vector.tensor_scalar_mul(qrRaw[:], ps[:RH], SCALE)
                            pr = pp.tile([RH, 512], F32, tag="mm", name="prq")
                            nc.tensor.matmul(pr[:], rt[:], qrRaw[:],
                                             start=True, stop=True)
                            nc.vector.tensor_mul(qrT[:], qrRaw[:], cosT[:, qsl])
                            rot = bcs.tile([RH, 512], F32, tag="rot",
                                           name="rotq")
                            nc.vector.tensor_mul(rot[:], pr[:], ssinT[:, qsl])
                            nc.vector.tensor_add(qrT[:], qrT[:], rot[:])

                    # ---- C chunk: causal attention for queries i4 ----
                    nj = 4 * i4 + 4
                    psden = pden.tile([1, 512], F32, tag="den", name="psden")
                    psy = [ppy.tile([P, 512], F32, tag=f"y{dc}",
                                    name=f"psy{dc}")
                           for dc in range(KV // P)]
                    for j in range(nj):
                        ksl = slice(j * P, (j + 1) * P)
                        ps = pp.tile([P, 512], F32, tag="mm", name="psS")
                        for dc in range(KV // P):
                            nc.tensor.matmul(
                                ps[:], ckvT[:, dc, ksl], qTc[dc][:],
                                start=(dc == 0), stop=False,
                            )
                        nc.tensor.matmul(
                            ps[:], krT2[:, ksl], qrT[:],
                            start=False, stop=True,
                        )
                        se = bcs.tile([P, 512], F32R, tag="se", bufs=3,
                                      name="se")
                        nc.scalar.activation(se[:], ps[:], AF.Exp)
                        r = j - 4 * i4
                        if r >= 0:
                            nc.vector.tensor_mul(se[:], se[:], masks[r][:])
                        nc.tensor.matmul(psden[:], ones_col[:], se[:],
                                         start=(j == 0), stop=(j == nj - 1))
                        for dc in range(KV // P):
                            nc.tensor.matmul(
                                psy[dc][:], v[:, j, dc * P : (dc + 1) * P],
                                se[:],
                                start=(j == 0), stop=(j == nj - 1),
                            )
                    deninv = bcs.tile([1, 512], F32, tag="deninv",
                                      name="deninv")
                    nc.vector.reciprocal_approx_fast(out=deninv[:],
                                                     in_=psden[:])
                    denb = bcs.tile([P, 512], F32, tag="denb", name="denb")
                    nc.gpsimd.partition_broadcast(denb[:], deninv[:])
                    for dc in range(KV // P):
                        yst = bcs.tile([P, 512], F32R, tag="yst", name="yst")
                        nc.vector.tensor_mul(yst[:], psy[dc][:], denb[:])
                        nc.scalar.dma_start(
                            yTd[h * (KV // P) + dc][:, qsl], yst[:]
                        )

        # ================= Phase D: out^T = W_out_c^T @ y^T ===============
        with ExitStack() as dctx:
            dyp = dctx.enter_context(tc.tile_pool(name="dyp", bufs=1))
            dwp = dctx.enter_context(tc.tile_pool(name="dwp", bufs=2))
            dst = dctx.enter_context(tc.tile_pool(name="dst", bufs=3))

            DK = HPC * KV // P  # 8 contraction chunks
            yT_sb = []
            for k in range(0, DK):
                yk = dyp.tile([P, T], F32R, tag=f"yT{k}", name=f"yT{k}")
                nc.gpsimd.dma_start(yk[:], yTd[k][:])
                yT_sb.append(yk)
            wo_r = wout.rearrange("(ko p) e -> p ko e", p=P)
            for mc in range(E // P):
                wo = dwp.tile([P, DK, P], F32R, tag="wo", name="wo")
                nc.sync.dma_start(wo[:], wo_r[:, :, mc * P : (mc + 1) * P])
                psD = [ppy.tile([P, 512], F32, tag=f"y{tcc}", name=f"psD{tcc}")
                       for tcc in range(NT512)]
                # kc-major: the stationary operand is reused across the 4
                # consecutive matmuls, letting LDWEIGHTS pull-ahead hide
                for kc in range(DK):
                    for tcc in range(NT512):
                        nc.tensor.matmul(
                            psD[tcc][:], wo[:, kc, :],
                            yT_sb[kc][:, tcc * 512 : (tcc + 1) * 512],
                            start=(kc == 0), stop=(kc == DK - 1),
                        )
                    if kc == DK - 1:
                        # copies chase the closing matmuls, split DVE/ACT
                        for tcc in range(NT512):
                            ost = dst.tile([P, 512], F32, tag=f"ost{tcc % 2}",
                                           name="ost")
                            if tcc % 2 == 0:
                                nc.vector.tensor_copy(ost[:], psD[tcc][:])
                            else:
                                nc.scalar.copy(ost[:], psD[tcc][:])
                            nc.scalar.dma_start(
                                outT[mc * P : (mc + 1) * P,
                                     tcc * 512 : (tcc + 1) * 512], ost[:]
                            )

    nc.compile()
    return nc


_NC_CACHE = {}


def _get_nc(T=T_FULL):
    if T not in _NC_CACHE:
        _NC_CACHE[T] = build_kernel(T)
    return _NC_CACHE[T]


def make_in_maps(x, cos, sin, W_qkv, W_qdec, W_out):
    """Host-side sharding/layout: transpose activations & tables, slice the
    head-parallel weights. Returns one input dict per core."""
    xT = np.ascontiguousarray(np.asarray(x)[0].T)
    cosT = np.ascontiguousarray(np.asarray(cos).T)
    sinT = np.ascontiguousarray(np.asarray(sin).T)
    W_qkv = np.ascontiguousarray(np.asarray(W_qkv))
    W_qdec = np.asarray(W_qdec)
    W_out = np.asarray(W_out)
    in_maps = []
    for c in range(NCORES):
        in_maps.append({
            "xT": xT,
            "wqkv": W_qkv,
            "wqdec": np.ascontiguousarray(
                W_qdec[:, c * HPC * QKH : (c + 1) * HPC * QKH]
            ),
            "wout": np.ascontiguousarray(
                W_out[c * HPC * KV : (c + 1) * HPC * KV]
            ),
            "cosT": cosT,
            "sinT": sinT,
        })
    return in_maps


def kernel(x, cos, sin, W_qkv, W_qdec, W_out, _trace=False, _tmpdir=None):
    T = np.asarray(x).shape[1]
    nc = _get_nc(T)
    in_maps = make_in_maps(x, cos, sin, W_qkv, W_qdec, W_out)
    res = run_bass_kernel_spmd(
        nc, in_maps, core_ids=list(range(NCORES)),
        trace=_trace, tmpdir=_tmpdir,
    )
    out = np.zeros((E, T), np.float32)
    for r in res.results:
        out += r["outT"]
    kernel.last_results = res
    return np.ascontiguousarray(out.T)[None].astype(np.float32)



# revision 3
# speedup vs baseline: 1.1290x; 1.1290x over previous
"""MLA-style causal self-attention on 8 Trainium2 NeuronCores.

Sharding: tensor-parallel over heads (2 heads/core) for phases B/C/D as
before, but phase A (latents = x @ W_qkv) is now sharded over T: each core
computes latents^T for its own 256-wide T-chunk (1/8 of the work), ropes
its local k_r slice, and an on-device AllGather (DRAM, bf16) replicates
the full latents to every core. All matmul operands are bf16 (same PE rate
as f32r on TRN2, half the DMA/SBUF), accumulation stays f32 in PSUM.
Each core returns a partial out^T [E, T] in bf16; the host sums in f32.

Device dataflow:
  A:    latents^T chunk = W_qkv^T @ x^T[:, own 256 cols]; 13 column
        groups -> bf16 -> DRAM bounce; k_r group roped in place first
        (rotate-half via 64x64 permutation matmul).
  AG:   AllGather [1600, 256] -> [8, 1600, 256]; readback DMAs place
        c_kv^T / roped k_r^T / c_q^T into canonical SBUF residents.
  preC: v = (c_kv^T)^T via PE transposes (bf16).
  B/C fused per head h and 512-query chunk i4 (as before): q^T chunk =
        W_qdec_h^T @ c_q^T (scale folded), rope group first, then
        flash-style causal attention; y^T stays SBUF-resident.
  D:    out^T = W_out_c^T @ y^T -> HBM in bf16 (kc-major stationary
        reuse).
"""

import math
from contextlib import ExitStack

import numpy as np

import concourse.bass as bass
import concourse.tile as tile
from concourse import bacc, mybir
from concourse.bass_utils import run_bass_kernel_spmd
from concourse.masks import make_identity

F32 = mybir.dt.float32
BF16 = mybir.dt.bfloat16
AF = mybir.ActivationFunctionType

# Problem constants (hardcoded per harness contract)
T_FULL = 2048
E = 2048          # n_embd
KV = 512          # kv low rank == head size
QL = 1024         # q low rank
RH = 64           # rope head size
QKH = KV + RH     # 576
NH = 16
NCORES = 8
HPC = NH // NCORES  # heads per core
SCALE = 1.0 / math.sqrt(float(KV))

P = 128
TC = T_FULL // NCORES  # 256, per-core T chunk for phase A


def _make_rot64(nc, pool):
    """RT [64, 64] permutation with RT[x, y] = 1 iff x == (y+32) % 64, so
    matmul(out, lhsT=RT, rhs=src) gives out[d] = src[(d+32) % 64]."""
    rt0 = pool.tile([RH, RH], F32, tag="rt0")
    nc.gpsimd.memset(rt0[:], 0.0)
    nc.gpsimd.affine_select(
        out=rt0[:], in_=rt0[:], compare_op=mybir.AluOpType.not_equal,
        fill=1.0, base=-32, channel_multiplier=1, pattern=[[-1, RH]],
    )
    nc.gpsimd.affine_select(
        out=rt0[:], in_=rt0[:], compare_op=mybir.AluOpType.not_equal,
        fill=1.0, base=32, channel_multiplier=1, pattern=[[-1, RH]],
    )
    rt = pool.tile([RH, RH], BF16, tag="rt")
    nc.vector.tensor_copy(rt[:], rt0[:])
    return rt


def build_kernel(T=T_FULL):
    """Build the single-core program (SPMD across 8 cores via per-core data)."""
    assert T % 512 == 0
    NT512 = T // 512
    NKT = T // P          # key tiles of 128
    EK = E // P           # 16 contraction chunks for phase A

    nc = bacc.Bacc("TRN2", target_bir_lowering=False, debug=False,
                   num_devices=NCORES)

    xT = nc.dram_tensor("xT", [E, TC], BF16, kind="ExternalInput").ap()
    wqkv = nc.dram_tensor("wqkv", [E, QKH + QL], BF16, kind="ExternalInput").ap()
    wqdec = nc.dram_tensor("wqdec", [QL, HPC * QKH], BF16, kind="ExternalInput").ap()
    wout = nc.dram_tensor("wout", [HPC * KV, E], BF16, kind="ExternalInput").ap()
    cosd = nc.dram_tensor("cosT", [RH, T], F32, kind="ExternalInput").ap()
    sind = nc.dram_tensor("sinT", [RH, T], F32, kind="ExternalInput").ap()
    # own-chunk rope tables (cols 256c..256c+256 of cosT/sinT, sin pre-negated
    # in the first half rows to fold the rotate-half sign)
    cosl = nc.dram_tensor("cosl", [RH, TC], F32, kind="ExternalInput").ap()
    sinl = nc.dram_tensor("sinl", [RH, TC], F32, kind="ExternalInput").ap()
    outT = nc.dram_tensor("outT", [E, T], BF16, kind="ExternalOutput").ap()

    xT_r = xT.rearrange("(ko p) t -> p ko t", p=P)
    wq_r = wqkv.rearrange("(ko p) m -> p ko m", p=P)

    # latent column groups: 4x128 c_kv, 1x64 k_r, 8x128 c_q
    groups = [(i * P, P) for i in range(KV // P)] + [(KV, RH)] + [
        (QKH + i * P, P) for i in range(QL // P)
    ]

    with tile.TileContext(nc) as tc, ExitStack() as ctx:
        dram = ctx.enter_context(tc.tile_pool(name="dram", bufs=1, space="DRAM"))
        cst = ctx.enter_context(tc.tile_pool(name="cst", bufs=1))
        kvp = ctx.enter_context(tc.tile_pool(name="kvp", bufs=1))
        pp = ctx.enter_context(tc.tile_pool(name="pp", bufs=2, space="PSUM"))
        ppy = ctx.enter_context(tc.tile_pool(name="ppy", bufs=1, space="PSUM"))

        latloc = dram.tile([QKH + QL, TC], BF16, tag="latloc")   # own latents^T
        latg = nc.dram_tensor(
            "latg", [NCORES, QKH + QL, TC], BF16, kind="Internal",
            addr_space="Shared",
        ).ap()

        # ---- global constants / kv residents ----
        ident0 = cst.tile([P, P], F32, tag="ident0")
        make_identity(nc, ident0[:])
        ident = cst.tile([P, P], BF16, tag="ident")
        nc.vector.tensor_copy(ident[:], ident0[:])
        ones0 = cst.tile([P, 1], F32, tag="ones0")
        nc.gpsimd.memset(ones0[:], 1.0)
        ones_col = cst.tile([P, 1], BF16, tag="ones")
        nc.vector.tensor_copy(ones_col[:], ones0[:])
        rt = _make_rot64(nc, cst)

        ckvT = kvp.tile([P, KV // P, T], BF16, tag="ckvT")  # c_kv^T (gathered)
        krT2 = kvp.tile([RH, T], BF16, tag="krT2")          # roped k_r^T
        cq = kvp.tile([P, QL // P, T], BF16, tag="cq")      # c_q^T (gathered)
        v = kvp.tile([P, NKT, KV], BF16, tag="v")           # c_kv [t, d]
        yT = kvp.tile([P, HPC * KV // P, T], BF16, tag="yT")  # y^T resident

        # ================= Phase A: own latents^T chunk ====================
        with ExitStack() as actx:
            aw = actx.enter_context(tc.tile_pool(name="aw", bufs=1))
            asr = actx.enter_context(tc.tile_pool(name="asr", bufs=1))
            astp = actx.enter_context(tc.tile_pool(name="astp", bufs=3))

            wqt = aw.tile([P, EK, QKH + QL], BF16, tag="wqt")
            # split the big weight load across queues
            nc.gpsimd.dma_start(wqt[:, 0 : EK // 2, :], wq_r[:, 0 : EK // 2, :])
            nc.scalar.dma_start(wqt[:, EK // 2 :, :], wq_r[:, EK // 2 :, :])
            xta = asr.tile([P, EK, TC], BF16, tag="xta")
            nc.sync.dma_start(xta[:], xT_r[:])
            cosls = asr.tile([RH, TC], F32, tag="cosls")
            nc.sync.dma_start(cosls[:], cosl[:])
            sinls = asr.tile([RH, TC], F32, tag="sinls")
            nc.sync.dma_start(sinls[:], sinl[:])

            for gi, (c0, M) in enumerate(groups):
                ps = pp.tile([P, TC], F32, tag="mm", name="psA")
                for kc in range(EK):
                    nc.tensor.matmul(
                        ps[:M], wqt[:, kc, c0 : c0 + M], xta[:, kc, :],
                        start=(kc == 0), stop=(kc == EK - 1),
                    )
                if c0 == KV:
                    # rope k_r in place before the gather
                    kr = astp.tile([RH, TC], BF16, tag="kr", name="kr")
                    nc.vector.tensor_copy(kr[:], ps[:RH])
                    pr = pp.tile([RH, TC], F32, tag="mm", name="prk")
                    nc.tensor.matmul(pr[:], rt[:], kr[:], start=True, stop=True)
                    kr2 = astp.tile([RH, TC], F32, tag="kr2", name="kr2")
                    nc.vector.tensor_mul(kr2[:], ps[:RH], cosls[:])
                    rot = astp.tile([RH, TC], F32, tag="rot", name="rotk")
                    nc.vector.tensor_mul(rot[:], pr[:], sinls[:])
                    kr2b = astp.tile([RH, TC], BF16, tag="kr2b", name="kr2b")
                    nc.vector.tensor_add(kr2b[:], kr2[:], rot[:])
                    nc.scalar.dma_start(latloc[c0 : c0 + M, :], kr2b[:])
                else:
                    st = astp.tile([P, TC], BF16, tag="ast", name="ast")
                    nc.vector.tensor_copy(st[:], ps[:])
                    nc.scalar.dma_start(latloc[c0 : c0 + M, :], st[:])

        # ================= AllGather + readback ===========================
        nc.gpsimd.collective_compute(
            "AllGather",
            mybir.AluOpType.bypass,
            replica_groups=[list(range(NCORES))],
            ins=[latloc[:]],
            outs=[latg[:]],
        )
        for r in range(NCORES):
            tsl = slice(r * TC, (r + 1) * TC)
            nc.sync.dma_start(
                cq[:, :, tsl],
                latg[r, QKH:, :].rearrange("(ko p) t -> p ko t", p=P),
            )
            nc.scalar.dma_start(
                ckvT[:, :, tsl],
                latg[r, :KV, :].rearrange("(dc p) t -> p dc t", p=P),
            )
            nc.gpsimd.dma_start(krT2[:, tsl], latg[r, KV:QKH, :])

        # ============ Phases preC + fused B/C =============================
        with ExitStack() as bctx:
            bcp = bctx.enter_context(tc.tile_pool(name="bcp", bufs=1))
            bcs = bctx.enter_context(tc.tile_pool(name="bcs", bufs=2))

            # edge masks for the diagonal 512-block
            masks = []
            for r in range(4):
                m0 = bcp.tile([P, 512], F32, tag=f"mask{r}", name=f"mask{r}")
                nc.gpsimd.memset(m0[:], 1.0)
                nc.gpsimd.affine_select(
                    out=m0[:], in_=m0[:], compare_op=mybir.AluOpType.is_ge,
                    fill=0.0, base=-P * r, channel_multiplier=-1,
                    pattern=[[1, 512]],
                )
                mb = bcp.tile([P, 512], BF16, tag=f"maskb{r}", name=f"maskb{r}")
                nc.vector.tensor_copy(mb[:], m0[:])
                masks.append(mb)
            cosT = bcp.tile([RH, T], F32, tag="cosT")
            nc.sync.dma_start(cosT[:], cosd[:])
            ssinT = bcp.tile([RH, T], F32, tag="ssinT")
            nc.sync.dma_start(ssinT[:], sind[:])

            # v[t, d] via PE transposes of c_kv^T (bf16)
            with tc.tile_pool(name="ptr", bufs=2, space="PSUM") as ptr:
                for dc in range(KV // P):
                    for tt in range(NKT):
                        pt = ptr.tile([P, P], BF16, tag="tr", name="pt")
                        nc.tensor.transpose(
                            pt[:], ckvT[:, dc, tt * P : (tt + 1) * P], ident[:]
                        )
                        nc.vector.tensor_copy(
                            v[:, tt, dc * P : (dc + 1) * P], pt[:]
                        )
            pden = bctx.enter_context(
                tc.tile_pool(name="pden", bufs=2, space="PSUM")
            )

            # rope group first so its chain overlaps the nope matmul groups
            qgroups = [(KV, RH)] + [(i * P, P) for i in range(KV // P)]
            for h in range(HPC):
                wqd = bcp.tile([P, QL // P, QKH], BF16, tag="wqd", name="wqd")
                nc.gpsimd.dma_start(
                    wqd[:],
                    wqdec.rearrange("(ko p) m -> p ko m", p=P)[
                        :, :, h * QKH : (h + 1) * QKH
                    ],
                )
                for i4 in range(NT512):
                    qsl = slice(i4 * 512, (i4 + 1) * 512)
                    # ---- B chunk: q^T for queries i4 (SCALE folded in) ----
                    qTc = [bcs.tile([P, 512], BF16, tag=f"qTc{i}",
                                    name=f"qTc{i}") for i in range(KV // P)]
                    qrRaw = bcs.tile([RH, 512], BF16, tag="qrRaw", name="qrRaw")
                    qrT = bcs.tile([RH, 512], BF16, tag="qrT", name="qrT")
                    for (m0, M) in qgroups:
                        ps = pp.tile([P, 512], F32, tag="mm", name="psB")
                        for kc in range(QL // P):
                            nc.tensor.matmul(
                                ps[:M], wqd[:, kc, m0 : m0 + M],
                                cq[:, kc, qsl],
                                start=(kc == 0), stop=(kc == QL // P - 1),
                            )
                        if m0 < KV:
                            nc.vector.tensor_scalar_mul(
                                qTc[m0 // P][:], ps[:], SCALE
                            )
                        else:
                            # rope group runs first; do its whole chain now so
                            # the DVE work hides under the four d-groups
                            nc.vector.tensor_scalar_mul(qrRaw[:], ps[:RH], SCALE)
                            pr = pp.tile([RH, 512], F32, tag="mm", name="prq")
                            nc.tensor.matmul(pr[:], rt[:], qrRaw[:],
                                             start=True, stop=True)
                            nc.vector.tensor_mul(qrT[:], qrRaw[:], cosT[:, qsl])
                            rot = bcs.tile([RH, 512], F32, tag="rot",
                                           name="rotq")
                            nc.vector.tensor_mul(rot[:], pr[:], ssinT[:, qsl])
                            nc.vector.tensor_add(qrT[:], qrT[:], rot[:])

                    # ---- C chunk: causal attention for queries i4 ----
                    nj = 4 * i4 + 4
                    psden = pden.tile([1, 512], F32, tag="den", name="psden")
                    psy = [ppy.tile([P, 512], F32, tag=f"y{dc}",
                                    name=f"psy{dc}")
                           for dc in range(KV // P)]
                    for j in range(nj):
                        ksl = slice(j * P, (j + 1) * P)
                        ps = pp.tile([P, 512], F32, tag="mm", name="psS")
                        for dc in range(KV // P):
                            nc.tensor.matmul(
                                ps[:], ckvT[:, dc, ksl], qTc[dc][:],
                                start=(dc == 0), stop=False,
                            )
                        nc.tensor.matmul(
                            ps[:], krT2[:, ksl], qrT[:],
                            start=False, stop=True,
                        )
                        se = bcs.tile([P, 512], BF16, tag="se", bufs=3,
                                      name="se")
                        nc.scalar.activation(se[:], ps[:], AF.Exp)
                        r = j - 4 * i4
                        if r >= 0:
                            nc.vector.tensor_mul(se[:], se[:], masks[r][:])
                        nc.tensor.matmul(psden[:], ones_col[:], se[:],
                                         start=(j == 0), stop=(j == nj - 1))
                        for dc in range(KV // P):
                            nc.tensor.matmul(
                                psy[dc][:], v[:, j, dc * P : (dc + 1) * P],
                                se[:],
                                start=(j == 0), stop=(j == nj - 1),
                            )
                    deninv = bcs.tile([1, 512], F32, tag="deninv",
                                      name="deninv")
                    nc.vector.reciprocal_approx_fast(out=deninv[:],
                                                     in_=psden[:])
                    denb = bcs.tile([P, 512], F32, tag="denb", name="denb")
                    nc.gpsimd.partition_broadcast(denb[:], deninv[:])
                    for dc in range(KV // P):
                        nc.vector.tensor_mul(
                            yT[:, h * (KV // P) + dc, qsl], psy[dc][:], denb[:]
                        )

        # ================= Phase D: out^T = W_out_c^T @ y^T ===============
        with ExitStack() as dctx:
            dwp = dctx.enter_context(tc.tile_pool(name="dwp", bufs=2))
            dst = dctx.enter_context(tc.tile_pool(name="dst", bufs=3))

            DK = HPC * KV // P  # 8 contraction chunks
            wo_r = wout.rearrange("(ko p) e -> p ko e", p=P)
            for mc in range(E // P):
                wo = dwp.tile([P, DK, P], BF16, tag="wo", name="wo")
                nc.sync.dma_start(wo[:], wo_r[:, :, mc * P : (mc + 1) * P])
                psD = [ppy.tile([P, 512], F32, tag=f"y{tcc}", name=f"psD{tcc}")
                       for tcc in range(NT512)]
                # kc-major: the stationary operand is reused across the 4
                # consecutive matmuls, letting LDWEIGHTS pull-ahead hide
                for kc in range(DK):
                    for tcc in range(NT512):
                        nc.tensor.matmul(
                            psD[tcc][:], wo[:, kc, :],
                            yT[:, kc, tcc * 512 : (tcc + 1) * 512],
                            start=(kc == 0), stop=(kc == DK - 1),
                        )
                    if kc == DK - 1:
                        # copies chase the closing matmuls, split DVE/ACT
                        for tcc in range(NT512):
                            ost = dst.tile([P, 512], BF16, tag=f"ost{tcc % 2}",
                                           name="ost")
                            if tcc % 2 == 0:
                                nc.vector.tensor_copy(ost[:], psD[tcc][:])
                            else:
                                nc.scalar.copy(ost[:], psD[tcc][:])
                            nc.scalar.dma_start(
                                outT[mc * P : (mc + 1) * P,
                                     tcc * 512 : (tcc + 1) * 512], ost[:]
                            )

    nc.compile()
    return nc


_NC_CACHE = {}


def _get_nc(T=T_FULL):
    if T not in _NC_CACHE:
        _NC_CACHE[T] = build_kernel(T)
    return _NC_CACHE[T]


def make_in_maps(x, cos, sin, W_qkv, W_qdec, W_out):
    """Host-side sharding/layout: transpose activations & tables, slice the
    head-parallel weights and the per-core T chunk of x. bf16 everywhere."""
    import ml_dtypes

    bf16 = ml_dtypes.bfloat16
    xT = np.ascontiguousarray(np.asarray(x)[0].T).astype(bf16)
    cosT = np.ascontiguousarray(np.asarray(cos).T.astype(np.float32))
    sinT = np.ascontiguousarray(np.asarray(sin).T.astype(np.float32))
    # pre-negate the first half rows of sin for the local k_r rope chain
    sinN = sinT.copy()
    sinN[: RH // 2, :] *= -1.0
    W_qkv = np.asarray(W_qkv).astype(bf16)
    W_qdec = np.asarray(W_qdec)
    W_out = np.asarray(W_out)
    in_maps = []
    for c in range(NCORES):
        tsl = slice(c * TC, (c + 1) * TC)
        in_maps.append({
            "xT": np.ascontiguousarray(xT[:, tsl]),
            "wqkv": W_qkv,
            "wqdec": np.ascontiguousarray(
                W_qdec[:, c * HPC * QKH : (c + 1) * HPC * QKH]
            ).astype(bf16),
            "wout": np.ascontiguousarray(
                W_out[c * HPC * KV : (c + 1) * HPC * KV]
            ).astype(bf16),
            "cosT": cosT,
            "sinT": _neg_first_half(sinT),
            "cosl": np.ascontiguousarray(cosT[:, tsl]),
            "sinl": np.ascontiguousarray(sinN[:, tsl]),
        })
    return in_maps


def _neg_first_half(sinT):
    s = sinT.copy()
    s[: RH // 2, :] *= -1.0
    return s


def kernel(x, cos, sin, W_qkv, W_qdec, W_out, _trace=False, _tmpdir=None):
    T = np.asarray(x).shape[1]
    nc = _get_nc(T)
    in_maps = make_in_maps(x, cos, sin, W_qkv, W_qdec, W_out)
    res = run_bass_kernel_spmd(
        nc, in_maps, core_ids=list(range(NCORES)),
        trace=_trace, tmpdir=_tmpdir,
    )
    out = np.zeros((E, T), np.float32)
    for r in res.results:
        out += np.asarray(r["outT"], dtype=np.float32)
    kernel.last_results = res
    return np.ascontiguousarray(out.T)[None].astype(np.float32)


# revision 8
# speedup vs baseline: 1.1706x; 1.0368x over previous
"""MLA-style causal self-attention on 8 Trainium2 NeuronCores.

Sharding: tensor-parallel over heads (2 heads/core) for phases B/C/D; phase A
(latents = x @ W_qkv) is sharded over T: each core computes latents^T for its
own 256-wide T-chunk, ropes its local k_r slice, and two on-device AllGathers
(DRAM->Shared DRAM, bf16) replicate the latents: the c_q gather goes first so
phase B can start while the (smaller) kv gather completes under it. All matmul
operands are bf16 (same PE rate as f32r on TRN2, half the DMA/SBUF),
accumulation stays f32 in PSUM. Each core returns a partial out^T (mc-major
[16, 128, 2048] bf16); the host sums in f32.

All weights are host-preswizzled to [128-partition, ko, m] layouts so every
DMA moves multi-KB contiguous per-partition lines (the DMA engines are
descriptor-rate-bound, ~70-145ns per packet regardless of size <= 4KB).
"""

import math
from contextlib import ExitStack

import numpy as np

import concourse.bass as bass
import concourse.tile as tile
from concourse import bacc, mybir
from concourse.bass_utils import run_bass_kernel_spmd
from concourse.masks import make_identity

F32 = mybir.dt.float32
BF16 = mybir.dt.bfloat16
AF = mybir.ActivationFunctionType

# Problem constants (hardcoded per harness contract)
T_FULL = 2048
E = 2048          # n_embd
KV = 512          # kv low rank == head size
QL = 1024         # q low rank
RH = 64           # rope head size
QKH = KV + RH     # 576
NH = 16
NCORES = 8
HPC = NH // NCORES  # heads per core
SCALE = 1.0 / math.sqrt(float(KV))

P = 128
TC = T_FULL // NCORES  # 256, per-core T chunk for phase A
QLC = QL // P          # 8 c_q row chunks
KVC = KV // P          # 4 c_kv row chunks
NKV = KVC + 1          # kv slots in the gather (4 c_kv + 1 padded rope)


def _make_rot64(nc, pool):
    """RT [64, 64] permutation with RT[x, y] = 1 iff x == (y+32) % 64, so
    matmul(out, lhsT=RT, rhs=src) gives out[d] = src[(d+32) % 64]."""
    rt0 = pool.tile([RH, RH], F32, tag="rt0")
    nc.gpsimd.memset(rt0[:], 0.0)
    nc.gpsimd.affine_select(
        out=rt0[:], in_=rt0[:], compare_op=mybir.AluOpType.not_equal,
        fill=1.0, base=-32, channel_multiplier=1, pattern=[[-1, RH]],
    )
    nc.gpsimd.affine_select(
        out=rt0[:], in_=rt0[:], compare_op=mybir.AluOpType.not_equal,
        fill=1.0, base=32, channel_multiplier=1, pattern=[[-1, RH]],
    )
    rt = pool.tile([RH, RH], BF16, tag="rt")
    nc.vector.tensor_copy(rt[:], rt0[:])
    return rt


def build_kernel(T=T_FULL):
    """Build the single-core program (SPMD across 8 cores via per-core data)."""
    assert T % 512 == 0
    NT512 = T // 512
    NKT = T // P          # key tiles of 128
    EK = E // P           # 16 contraction chunks for phase A
    EH = EK // 2

    nc = bacc.Bacc("TRN2", target_bir_lowering=False, debug=False,
                   num_devices=NCORES)

    # host-preswizzled inputs: [p, ko, m] with ko the contraction chunk
    xT = nc.dram_tensor("xT", [P, EK, TC], BF16, kind="ExternalInput").ap()
    wcq = nc.dram_tensor("wcq", [P, EK, QL], BF16, kind="ExternalInput").ap()
    wkv = nc.dram_tensor("wkv", [P, EK, QKH], BF16, kind="ExternalInput").ap()
    wqd = nc.dram_tensor("wqd", [P, QLC, HPC * QKH], BF16,
                         kind="ExternalInput").ap()
    wo = nc.dram_tensor("wo", [E // P, P, HPC * KV // P, P], BF16,
                        kind="ExternalInput").ap()
    cosd = nc.dram_tensor("cosT", [RH, T], BF16, kind="ExternalInput").ap()
    sind = nc.dram_tensor("sinT", [RH, T], BF16, kind="ExternalInput").ap()
    cosl = nc.dram_tensor("cosl", [RH, TC], F32, kind="ExternalInput").ap()
    sinl = nc.dram_tensor("sinl", [RH, TC], F32, kind="ExternalInput").ap()
    outT = nc.dram_tensor("outT", [E // P, P, T], BF16,
                          kind="ExternalOutput").ap()

    with tile.TileContext(nc) as tc, ExitStack() as ctx:
        dram = ctx.enter_context(tc.tile_pool(name="dram", bufs=1, space="DRAM"))
        cst = ctx.enter_context(tc.tile_pool(name="cst", bufs=1))
        kvp = ctx.enter_context(tc.tile_pool(name="kvp", bufs=1))
        pp = ctx.enter_context(tc.tile_pool(name="pp", bufs=2, space="PSUM"))
        ppy = ctx.enter_context(tc.tile_pool(name="ppy", bufs=1, space="PSUM"))

        latloc_cq = dram.tile([P, QLC, TC], BF16, tag="latcq")
        latloc_kv = dram.tile([P, NKV, TC], BF16, tag="latkv")
        latg_cq = nc.dram_tensor("latg_cq", [NCORES, P, QLC, TC], BF16,
                                 kind="Internal", addr_space="Shared").ap()
        latg_kv = nc.dram_tensor("latg_kv", [NCORES, P, NKV, TC], BF16,
                                 kind="Internal", addr_space="Shared").ap()

        # ---- global constants (built early so gpsimd queue is clear when
        # the collectives become ready) ----
        ident0 = cst.tile([P, P], F32, tag="ident0")
        make_identity(nc, ident0[:])
        ident = cst.tile([P, P], BF16, tag="ident")
        nc.vector.tensor_copy(ident[:], ident0[:])
        ones0 = cst.tile([P, 1], F32, tag="ones0")
        nc.gpsimd.memset(ones0[:], 1.0)
        ones_col = cst.tile([P, 1], BF16, tag="ones")
        nc.vector.tensor_copy(ones_col[:], ones0[:])
        rt = _make_rot64(nc, cst)
        masks = []
        with tc.tile_pool(name="mtmp", bufs=1) as mtmp:
            for r in range(4):
                m0 = mtmp.tile([P, 512], F32, tag=f"mask{r}", name=f"mask{r}")
                nc.gpsimd.memset(m0[:], 1.0)
                nc.gpsimd.affine_select(
                    out=m0[:], in_=m0[:], compare_op=mybir.AluOpType.is_ge,
                    fill=0.0, base=-P * r, channel_multiplier=-1,
                    pattern=[[1, 512]],
                )
                mb = cst.tile([P, 512], BF16, tag=f"maskb{r}",
                              name=f"maskb{r}")
                nc.vector.tensor_copy(mb[:], m0[:])
                masks.append(mb)

        # ================= Phase A: own latents^T chunk ====================
        with ExitStack() as actx:
            aw = actx.enter_context(tc.tile_pool(name="aw", bufs=1))
            asr = actx.enter_context(tc.tile_pool(name="asr", bufs=1))
            astp = actx.enter_context(tc.tile_pool(name="astp", bufs=2))

            # c_q weight halves first (cq gather is the critical path)
            wcqa = aw.tile([P, EH, QL], BF16, tag="wcqa")
            nc.gpsimd.dma_start(wcqa[:], wcq[:, 0:EH, :])
            wcqb = aw.tile([P, EH, QL], BF16, tag="wcqb")
            nc.scalar.dma_start(wcqb[:], wcq[:, EH:EK, :])
            xta = asr.tile([P, EH, TC], BF16, tag="xta")
            nc.sync.dma_start(xta[:], xT[:, 0:EH, :])
            xtb = asr.tile([P, EH, TC], BF16, tag="xtb")
            nc.sync.dma_start(xtb[:], xT[:, EH:EK, :])
            wkva = aw.tile([P, EH, QKH], BF16, tag="wkva")
            nc.gpsimd.dma_start(wkva[:], wkv[:, 0:EH, :])
            wkvb = aw.tile([P, EH, QKH], BF16, tag="wkvb")
            nc.scalar.dma_start(wkvb[:], wkv[:, EH:EK, :])
            cosls = asr.tile([RH, TC], F32, tag="cosls")
            nc.sync.dma_start(cosls[:], cosl[:])
            sinls = asr.tile([RH, TC], F32, tag="sinls")
            nc.sync.dma_start(sinls[:], sinl[:])

            st_cq = asr.tile([P, QLC, TC], BF16, tag="st_cq")
            for gi in range(QLC):
                c0 = gi * P
                ps = pp.tile([P, TC], F32, tag="mm", name="psA")
                for kc in range(EK):
                    wt = (wcqa[:, kc, c0 : c0 + P] if kc < EH
                          else wcqb[:, kc - EH, c0 : c0 + P])
                    xs = xta[:, kc, :] if kc < EH else xtb[:, kc - EH, :]
                    nc.tensor.matmul(ps[:], wt, xs,
                                     start=(kc == 0), stop=(kc == EK - 1))
                nc.vector.tensor_copy(st_cq[:, gi, :], ps[:])
            nc.scalar.dma_start(latloc_cq[:], st_cq[:])
            nc.gpsimd.collective_compute(
                "AllGather", mybir.AluOpType.bypass,
                replica_groups=[list(range(NCORES))],
                ins=[latloc_cq[:]], outs=[latg_cq[:]],
            )

            st_kv = asr.tile([P, NKV, TC], BF16, tag="st_kv")
            for dc in range(KVC):
                c0 = dc * P
                ps = pp.tile([P, TC], F32, tag="mm", name="psA")
                for kc in range(EK):
                    wt = (wkva[:, kc, c0 : c0 + P] if kc < EH
                          else wkvb[:, kc - EH, c0 : c0 + P])
                    xs = xta[:, kc, :] if kc < EH else xtb[:, kc - EH, :]
                    nc.tensor.matmul(ps[:], wt, xs,
                                     start=(kc == 0), stop=(kc == EK - 1))
                nc.vector.tensor_copy(st_kv[:, dc, :], ps[:])
            # rope group: rotate-half via permutation matmul, local tables
            ps = pp.tile([P, TC], F32, tag="mm", name="psA")
            for kc in range(EK):
                wt = (wkva[:, kc, KV : KV + RH] if kc < EH
                      else wkvb[:, kc - EH, KV : KV + RH])
                xs = xta[:, kc, :] if kc < EH else xtb[:, kc - EH, :]
                nc.tensor.matmul(ps[:RH], wt, xs,
                                 start=(kc == 0), stop=(kc == EK - 1))
            kr = astp.tile([RH, TC], BF16, tag="kr", name="kr")
            nc.vector.tensor_copy(kr[:], ps[:RH])
            pr = pp.tile([RH, TC], F32, tag="mm", name="prk")
            nc.tensor.matmul(pr[:], rt[:], kr[:], start=True, stop=True)
            kr2 = astp.tile([RH, TC], F32, tag="kr2", name="kr2")
            nc.vector.tensor_mul(kr2[:], ps[:RH], cosls[:])
            rot = astp.tile([RH, TC], F32, tag="rot", name="rotk")
            nc.vector.tensor_mul(rot[:], pr[:], sinls[:])
            nc.vector.tensor_add(st_kv[:RH, KVC, :], kr2[:], rot[:])
            nc.scalar.dma_start(latloc_kv[:], st_kv[:])
            nc.gpsimd.collective_compute(
                "AllGather", mybir.AluOpType.bypass,
                replica_groups=[list(range(NCORES))],
                ins=[latloc_kv[:]], outs=[latg_kv[:]],
            )

        # ---- residents for B/C/D (allocated after phase A frees its pools;
        # the DMAs load during the gather window) ----
        cq_sb = kvp.tile([P, NCORES, QLC, TC], BF16, tag="cq_sb")
        kvg_sb = kvp.tile([P, NCORES, NKV, TC], BF16, tag="kvg_sb")
        v = kvp.tile([P, NKT, KV], BF16, tag="v")           # c_kv [t, d]
        yT = kvp.tile([P, HPC * KV // P, T], BF16, tag="yT")  # y^T resident
        cosT = kvp.tile([RH, T], BF16, tag="cosT")
        nc.sync.dma_start(cosT[:], cosd[:])
        ssinT = kvp.tile([RH, T], BF16, tag="ssinT")
        nc.sync.dma_start(ssinT[:], sind[:])
        wqds = kvp.tile([P, QLC, HPC * QKH], BF16, tag="wqds")
        nc.scalar.dma_start(wqds[:], wqd[:])

        # ================= readback (big-line DMAs) =======================
        for i in range(4):
            nc.sync.dma_start(
                cq_sb[:, 2 * i : 2 * i + 2, :, :],
                latg_cq[2 * i : 2 * i + 2].rearrange("r p ko t -> p r ko t"),
            )
        for i in range(2):
            nc.scalar.dma_start(
                kvg_sb[:, 4 * i : 4 * i + 4, :, :],
                latg_kv[4 * i : 4 * i + 4].rearrange("r p g t -> p r g t"),
            )

        # ============ Phases preC + fused B/C =============================
        with ExitStack() as bctx:
            bcs = bctx.enter_context(tc.tile_pool(name="bcs", bufs=2))

            # v[t, d] via PE transposes of c_kv^T (bf16), key-tile-major so
            # early tiles are ready for C(h0, i4=0)
            with tc.tile_pool(name="ptr", bufs=2, space="PSUM") as ptr:
                for tt in range(NKT):
                    r, hh = tt // 2, tt % 2
                    for dc in range(KVC):
                        pt = ptr.tile([P, P], BF16, tag="tr", name="pt")
                        nc.tensor.transpose(
                            pt[:], kvg_sb[:, r, dc, hh * P : (hh + 1) * P],
                            ident[:],
                        )
                        nc.vector.tensor_copy(
                            v[:, tt, dc * P : (dc + 1) * P], pt[:]
                        )
            pden = bctx.enter_context(
                tc.tile_pool(name="pden", bufs=2, space="PSUM")
            )

            # rope group first so its chain overlaps the nope matmul groups
            qgroups = [(KV, RH)] + [(i * P, P) for i in range(KVC)]
            for h in range(HPC):
                for i4 in range(NT512):
                    qsl = slice(i4 * 512, (i4 + 1) * 512)
                    # ---- B chunk: q^T for queries i4 (SCALE folded in) ----
                    qTc = [bcs.tile([P, 512], BF16, tag=f"qTc{i}",
                                    name=f"qTc{i}") for i in range(KVC)]
                    qrRaw = bcs.tile([RH, 512], BF16, tag="qrRaw", name="qrRaw")
                    qrT = bcs.tile([RH, 512], BF16, tag="qrT", name="qrT")
                    for (m0, M) in qgroups:
                        ps = pp.tile([P, 512], F32, tag="mm", name="psB")
                        for kc in range(QLC):
                            nc.tensor.matmul(
                                ps[:M], wqds[:, kc, h * QKH + m0 :
                                             h * QKH + m0 + M],
                                cq_sb[:, 2 * i4 : 2 * i4 + 2, kc, :],
                                start=(kc == 0), stop=(kc == QLC - 1),
                            )
                        if m0 < KV:
                            nc.vector.tensor_scalar_mul(
                                qTc[m0 // P][:], ps[:], SCALE
                            )
                        else:
                            nc.vector.tensor_scalar_mul(qrRaw[:], ps[:RH], SCALE)
                            pr = pp.tile([RH, 512], F32, tag="mm", name="prq")
                            nc.tensor.matmul(pr[:], rt[:], qrRaw[:],
                                             start=True, stop=True)
                            nc.vector.tensor_mul(qrT[:], qrRaw[:], cosT[:, qsl])
                            rot = bcs.tile([RH, 512], F32, tag="rot",
                                           name="rotq")
                            nc.vector.tensor_mul(rot[:], pr[:], ssinT[:, qsl])
                            nc.vector.tensor_add(qrT[:], qrT[:], rot[:])

                    # ---- C chunk: causal attention for queries i4 ----
                    nj = 4 * i4 + 4
                    psden = pden.tile([1, 512], F32, tag="den", name="psden")
                    psy = [ppy.tile([P, 512], F32, tag=f"y{dc}",
                                    name=f"psy{dc}")
                           for dc in range(KVC)]
                    for j in range(nj):
                        jr, jh = j // 2, j % 2
                        jsl = slice(jh * P, (jh + 1) * P)
                        ps = pp.tile([P, 512], F32, tag="mm", name="psS")
                        for dc in range(KVC):
                            nc.tensor.matmul(
                                ps[:], kvg_sb[:, jr, dc, jsl], qTc[dc][:],
                                start=(dc == 0), stop=False,
                            )
                        nc.tensor.matmul(
                            ps[:], kvg_sb[:RH, jr, KVC, jsl], qrT[:],
                            start=False, stop=True,
                        )
                        se = bcs.tile([P, 512], BF16, tag="se", bufs=3,
                                      name="se")
                        nc.scalar.activation(se[:], ps[:], AF.Exp)
                        r = j - 4 * i4
                        if r >= 0:
                            nc.vector.tensor_mul(se[:], se[:], masks[r][:])
                        nc.tensor.matmul(psden[:], ones_col[:], se[:],
                                         start=(j == 0), stop=(j == nj - 1))
                        for dc in range(KVC):
                            nc.tensor.matmul(
                                psy[dc][:], v[:, j, dc * P : (dc + 1) * P],
                                se[:],
                                start=(j == 0), stop=(j == nj - 1),
                            )
                    deninv = bcs.tile([1, 512], F32, tag="deninv",
                                      name="deninv")
                    nc.vector.reciprocal_approx_fast(out=deninv[:],
                                                     in_=psden[:])
                    denb = bcs.tile([P, 512], F32, tag="denb", name="denb")
                    nc.gpsimd.partition_broadcast(denb[:], deninv[:])
                    for dc in range(KVC):
                        nc.vector.tensor_mul(
                            yT[:, h * KVC + dc, qsl], psy[dc][:], denb[:]
                        )

        # ================= Phase D: out^T = W_out_c^T @ y^T ===============
        with ExitStack() as dctx:
            dwp = dctx.enter_context(tc.tile_pool(name="dwp", bufs=2))
            dst = dctx.enter_context(tc.tile_pool(name="dst", bufs=2))

            DK = HPC * KV // P  # 8 contraction chunks
            for mc in range(E // P):
                wot = dwp.tile([P, DK, P], BF16, tag="wo", name="wo")
                nc.sync.dma_start(wot[:], wo[mc])
                psD = [ppy.tile([P, 512], F32, tag=f"y{tcc}", name=f"psD{tcc}")
                       for tcc in range(NT512)]
                # kc-major: the stationary operand is reused across the 4
                # consecutive matmuls, letting LDWEIGHTS pull-ahead hide
                for kc in range(DK):
                    for tcc in range(NT512):
                        nc.tensor.matmul(
                            psD[tcc][:], wot[:, kc, :],
                            yT[:, kc, tcc * 512 : (tcc + 1) * 512],
                            start=(kc == 0), stop=(kc == DK - 1),
                        )
                ost = dst.tile([P, T], BF16, tag="ost", name="ost")
                for tcc in range(NT512):
                    osl = slice(tcc * 512, (tcc + 1) * 512)
                    if tcc % 2 == 0:
                        nc.vector.tensor_copy(ost[:, osl], psD[tcc][:])
                    else:
                        nc.scalar.copy(ost[:, osl], psD[tcc][:])
                eng = nc.gpsimd if mc % 2 == 0 else nc.scalar
                eng.dma_start(outT[mc], ost[:])

    nc.compile()
    return nc


_NC_CACHE = {}


def _get_nc(T=T_FULL):
    if T not in _NC_CACHE:
        _NC_CACHE[T] = build_kernel(T)
    return _NC_CACHE[T]


def _swizzle_k(w, p=P):
    """[K, M] -> [p, K//p, M] with row k = ko*p + pp."""
    K, M = w.shape
    return np.ascontiguousarray(w.reshape(K // p, p, M).transpose(1, 0, 2))


def make_in_maps(x, cos, sin, W_qkv, W_qdec, W_out):
    """Host-side sharding/layout: transpose activations & tables, slice the
    head-parallel weights and the per-core T chunk of x; preswizzle every
    weight to [p, ko, m]; bf16 everywhere."""
    import ml_dtypes

    bf16 = ml_dtypes.bfloat16
    xT = np.ascontiguousarray(np.asarray(x)[0].T).astype(bf16)
    cosT = np.ascontiguousarray(np.asarray(cos).T.astype(np.float32))
    sinT = np.ascontiguousarray(np.asarray(sin).T.astype(np.float32))
    sinN = sinT.copy()
    sinN[: RH // 2, :] *= -1.0
    W_qkv = np.asarray(W_qkv).astype(bf16)
    wcq = _swizzle_k(W_qkv[:, QKH:])
    wkv = _swizzle_k(W_qkv[:, :QKH])
    W_qdec = np.asarray(W_qdec)
    W_out = np.asarray(W_out)
    in_maps = []
    for c in range(NCORES):
        tsl = slice(c * TC, (c + 1) * TC)
        wos = W_out[c * HPC * KV : (c + 1) * HPC * KV].astype(bf16)
        wos = _swizzle_k(wos)                      # [128, 8, 2048]
        wos = wos.reshape(P, HPC * KV // P, E // P, P)
        wos = np.ascontiguousarray(wos.transpose(2, 0, 1, 3))  # [16,128,8,128]
        in_maps.append({
            "xT": _swizzle_k(np.ascontiguousarray(xT[:, tsl])),
            "wcq": wcq,
            "wkv": wkv,
            "wqd": _swizzle_k(np.ascontiguousarray(
                W_qdec[:, c * HPC * QKH : (c + 1) * HPC * QKH]).astype(bf16)),
            "wo": wos,
            "cosT": cosT.astype(bf16),
            "sinT": sinN.astype(bf16),
            "cosl": np.ascontiguousarray(cosT[:, tsl]),
            "sinl": np.ascontiguousarray(sinN[:, tsl]),
        })
    return in_maps


def kernel(x, cos, sin, W_qkv, W_qdec, W_out, _trace=False, _tmpdir=None):
    T = np.asarray(x).shape[1]
    nc = _get_nc(T)
    in_maps = make_in_maps(x, cos, sin, W_qkv, W_qdec, W_out)
    res = run_bass_kernel_spmd(
        nc, in_maps, core_ids=list(range(NCORES)),
        trace=_trace, tmpdir=_tmpdir,
    )
    out = np.zeros((E, T), np.float32)
    for r in res.results:
        out += np.asarray(r["outT"], dtype=np.float32).reshape(E, T)
    kernel.last_results = res
    return np.ascontiguousarray(out.T)[None].astype(np.float32)
